# revision 6
# baseline (speedup 1.0000x reference)
"""Trainium2 Bass kernel for DTDRLinear.

Reference computation:
    y = hadamard(x) @ (Q.astype(f32) * s).T + bias        x:[4,2048,4096]

Key algebra: the 4096-point normalized Walsh-Hadamard transform is the
symmetric Sylvester matrix H4096/64, and H4096 = H32 (x) H128 (Kronecker,
index i = a*128 + b).  So with x kept transposed (in_f on partitions):

    z = H4096 @ x^T      (unnormalized)
      = (stage 1)  per 128-row chunk a:  u_a = H128 @ xT_a     -> PE matmul
        (stage 2)  5 butterfly stages over the 32 chunk indices -> DVE adds
    y^T[o,:] = (s[o]/64) * (Q[o,:] @ z) + bias[o]              -> PE + ACT

Sharding: data-parallel over the 8192 tokens across 8 cores (1024 each);
Q/s/bias replicated.  The dequant scale s/64 and bias are fused into the
PSUM->SBUF eviction on the Scalar engine, so the weights used on the PE are
the raw int8 values of Q exactly represented in bf16.
"""

import math
import os
import sys

import numpy as np

for _p in ("/opt/trn_rl_repo", os.path.expanduser("~/.axon_site/_ro/trn_rl_repo")):
    if os.path.isdir(_p) and _p not in sys.path:
        sys.path.insert(0, _p)

import ml_dtypes  # noqa: E402

BF16 = ml_dtypes.bfloat16

# Problem shape (hardcoded per the task contract).
B, S, IN_F, OUT_F = 4, 2048, 4096, 4096
N_CORES = 8
M_TOT = B * S                    # 8192 tokens
M_CORE = M_TOT // N_CORES        # 1024 tokens per core
M_HALF = M_CORE // 2             # 512-token halves pipelined through SBUF
P = 128
A_CH = IN_F // P                 # 32 chunks of 128 along in_f
O_CH = OUT_F // P                # 32 tiles of 128 along out_f

_CACHE: dict = {}
LAST_RESULTS = None              # BassKernelResults from the most recent run


def _sylvester(k: int) -> np.ndarray:
    H = np.array([[1.0]], dtype=np.float32)
    for _ in range(k):
        H = np.block([[H, H], [H, -H]])
    return H


def _build_nc():
    import concourse.mybir as mybir
    import concourse.tile as tile
    from concourse import bacc

    bf = mybir.dt.bfloat16
    f32 = mybir.dt.float32
    IDENT = mybir.ActivationFunctionType.Identity

    nc = bacc.Bacc("TRN2", target_bir_lowering=False, debug=False,
                   enable_asserts=False, enable_partition_id=False)

    xT = nc.dram_tensor("xT", [A_CH, P, M_CORE], bf, kind="ExternalInput").ap()
    wT = nc.dram_tensor("wT", [O_CH, P, A_CH, P], bf, kind="ExternalInput").ap()
    sb = nc.dram_tensor("sb", [P, O_CH], f32, kind="ExternalInput").ap()
    bb = nc.dram_tensor("bb", [P, O_CH], f32, kind="ExternalInput").ap()
    hh = nc.dram_tensor("h", [P, P], bf, kind="ExternalInput").ap()
    yT = nc.dram_tensor("yT", [O_CH, P, M_CORE], f32, kind="ExternalOutput").ap()

    with tile.TileContext(nc) as tc:
        with (
            tc.tile_pool(name="const", bufs=1) as cpool,
            tc.tile_pool(name="xin", bufs=4) as xpool,
            tc.tile_pool(name="zbuf", bufs=1) as zpool,
            tc.tile_pool(name="wts", bufs=3) as wpool,
            tc.tile_pool(name="outs", bufs=4) as opool,
            tc.tile_pool(name="ps1", bufs=2, space="PSUM") as ps1,
            tc.tile_pool(name="psmm", bufs=4, space="PSUM") as psmm,
        ):
            h_t = cpool.tile([P, P], bf, tag="h")
            nc.sync.dma_start(h_t, hh)
            sb_t = cpool.tile([P, O_CH], f32, tag="sb")
            nc.sync.dma_start(sb_t, sb)
            bb_t = cpool.tile([P, O_CH], f32, tag="bb")
            nc.sync.dma_start(bb_t, bb)

            def stage1(mh, ping):
                msl = slice(mh * M_HALF, (mh + 1) * M_HALF)
                for a in range(A_CH):
                    x_t = xpool.tile([P, M_HALF], bf, tag="x")
                    nc.sync.dma_start(x_t, xT[a, :, msl])
                    ps = ps1.tile([P, M_HALF], f32, tag="s1")
                    nc.tensor.matmul(ps, h_t, x_t, start=True, stop=True)
                    nc.scalar.copy(ping[:, a, :], ps)

            def butterflies(ping, pong):
                src, dst = ping, pong
                for st in range(5):
                    hstep = 1 << st
                    for i in range(A_CH):
                        if i & hstep:
                            continue
                        j = i + hstep
                        nc.vector.tensor_add(dst[:, i, :], src[:, i, :], src[:, j, :])
                        nc.vector.tensor_sub(dst[:, j, :], src[:, i, :], src[:, j, :])
                    src, dst = dst, src
                return src  # buffer holding the final transform

            def gemm(mh, zfin):
                msl = slice(mh * M_HALF, (mh + 1) * M_HALF)
                for ot in range(O_CH):
                    w_t = wpool.tile([P, A_CH, P], bf, tag="w")
                    nc.sync.dma_start(w_t, wT[ot])
                    ps = psmm.tile([P, M_HALF], f32, tag="mm")
                    for k in range(A_CH):
                        nc.tensor.matmul(ps, w_t[:, k, :], zfin[:, k, :],
                                         start=(k == 0), stop=(k == A_CH - 1))
                    o_t = opool.tile([P, M_HALF], f32, tag="o")
                    nc.scalar.activation(o_t, ps, IDENT,
                                         bias=bb_t[:, ot:ot + 1],
                                         scale=sb_t[:, ot:ot + 1])
                    nc.sync.dma_start(yT[ot, :, msl], o_t)

            zfin = []
            for mh in range(2):
                ping = zpool.tile([P, A_CH, M_HALF], bf, tag="ping")
                pong = zpool.tile([P, A_CH, M_HALF], bf, tag=f"pong{mh}")
                stage1(mh, ping)
                zfin.append(butterflies(ping, pong))
            for mh in range(2):
                gemm(mh, zfin[mh])

    nc.compile()
    return nc


def _prep_shared(Q, s, bias):
    """Host-side packaging of the replicated operands (layout + dtype only)."""
    Q = np.asarray(Q)
    # Q arrives int8 (reference setup) or int32 (spec); both exact in bf16.
    q4 = Q.reshape(O_CH, P, A_CH, P)            # [ot, o, k, p]
    w_host = np.ascontiguousarray(
        q4.transpose(0, 3, 2, 1)).astype(BF16)  # [ot, p, k, o]
    s_eff = (np.asarray(s, dtype=np.float32).reshape(OUT_F) / 64.0)
    sb_host = np.ascontiguousarray(s_eff.reshape(O_CH, P).T)         # [p, ot]
    bb_host = np.ascontiguousarray(
        np.asarray(bias, dtype=np.float32).reshape(O_CH, P).T)       # [p, ot]
    h_host = _sylvester(7).astype(BF16)                              # [128,128]
    return w_host, sb_host, bb_host, h_host


def _make_in_maps(x, Q, s, bias):
    x = np.asarray(x, dtype=np.float32)
    w_host, sb_host, bb_host, h_host = _prep_shared(Q, s, bias)
    x_flat = x.reshape(M_TOT, IN_F).astype(BF16)
    in_maps = []
    for c in range(N_CORES):
        shard = x_flat[c * M_CORE:(c + 1) * M_CORE]              # [1024, 4096]
        xT_host = np.ascontiguousarray(shard.T).reshape(A_CH, P, M_CORE)
        in_maps.append({
            "xT": xT_host,
            "wT": w_host,
            "sb": sb_host,
            "bb": bb_host,
            "h": h_host,
        })
    return in_maps


def _assemble_output(results):
    y = np.empty((M_TOT, OUT_F), dtype=np.float32)
    for c in range(N_CORES):
        yT_core = results[c]["yT"]                               # [32,128,1024]
        y[c * M_CORE:(c + 1) * M_CORE] = (
            yT_core.transpose(2, 0, 1).reshape(M_CORE, OUT_F))
    return y.reshape(B, S, OUT_F)


def kernel(x, Q, s, bias):
    global LAST_RESULTS
    from concourse.bass_utils import run_bass_kernel_spmd

    if "nc" not in _CACHE:
        _CACHE["nc"] = _build_nc()
    nc = _CACHE["nc"]

    in_maps = _make_in_maps(x, Q, s, bias)
    res = run_bass_kernel_spmd(nc, in_maps, core_ids=list(range(N_CORES)))
    LAST_RESULTS = res
    return _assemble_output(res.results)


# revision 8
# speedup vs baseline: 4.7052x; 4.7052x over previous
"""Trainium2 Bass kernel for DTDRLinear.

Reference computation:
    y = hadamard(x) @ (Q.astype(f32) * s).T + bias        x:[4,2048,4096]

Key algebra: the 4096-point normalized Walsh-Hadamard transform is the
symmetric Sylvester matrix H4096/64, and H4096 = H32 (x) H128 (Kronecker,
index i = a*128 + b).  So with x kept transposed (in_f on partitions):

    z = H4096 @ x^T      (unnormalized)
      = (stage 1)  per 128-row chunk a:  u_a = H128 @ xT_a     -> PE matmul
        (stage 2)  5 butterfly stages over the 32 chunk indices -> DVE adds
    y^T[o,:] = (s[o]/64) * (Q[o,:] @ z) + bias[o]              -> PE + ACT

Sharding: data-parallel over the 8192 tokens across 8 cores (1024 each);
Q/s/bias replicated.  The dequant scale s/64 and bias are fused into the
PSUM->SBUF eviction on the Scalar engine, so the weights used on the PE are
the raw int8 values of Q exactly represented in bf16.
"""

import math
import os
import sys

import numpy as np

for _p in ("/opt/trn_rl_repo", os.path.expanduser("~/.axon_site/_ro/trn_rl_repo")):
    if os.path.isdir(_p) and _p not in sys.path:
        sys.path.insert(0, _p)

import ml_dtypes  # noqa: E402

BF16 = ml_dtypes.bfloat16

# Problem shape (hardcoded per the task contract).
B, S, IN_F, OUT_F = 4, 2048, 4096, 4096
N_CORES = 8
M_TOT = B * S                    # 8192 tokens
M_CORE = M_TOT // N_CORES        # 1024 tokens per core
M_HALF = M_CORE // 2             # 512-token halves pipelined through SBUF
P = 128
A_CH = IN_F // P                 # 32 chunks of 128 along in_f
O_CH = OUT_F // P                # 32 tiles of 128 along out_f

_CACHE: dict = {}
LAST_RESULTS = None              # BassKernelResults from the most recent run


def _sylvester(k: int) -> np.ndarray:
    H = np.array([[1.0]], dtype=np.float32)
    for _ in range(k):
        H = np.block([[H, H], [H, -H]])
    return H


def _build_nc(rep: int = 1):
    """Build the SPMD per-core program. rep>1 repeats the whole compute body
    (same inputs, overwriting outputs) for amplified wall-clock timing."""
    import concourse.mybir as mybir
    import concourse.tile as tile
    from concourse import bacc

    bf = mybir.dt.bfloat16
    f32 = mybir.dt.float32
    IDENT = mybir.ActivationFunctionType.Identity

    nc = bacc.Bacc("TRN2", target_bir_lowering=False, debug=False,
                   enable_asserts=False, enable_partition_id=False)

    xT = nc.dram_tensor("xT", [A_CH, P, M_CORE], bf, kind="ExternalInput").ap()
    wT = nc.dram_tensor("wT", [O_CH, P, A_CH, P], bf, kind="ExternalInput").ap()
    sb = nc.dram_tensor("sb", [P, O_CH], f32, kind="ExternalInput").ap()
    bb = nc.dram_tensor("bb", [P, O_CH], f32, kind="ExternalInput").ap()
    hh = nc.dram_tensor("h", [P, P], bf, kind="ExternalInput").ap()
    yT = nc.dram_tensor("yT", [O_CH, P, M_CORE], f32, kind="ExternalOutput").ap()

    with tile.TileContext(nc) as tc:
        with (
            tc.tile_pool(name="const", bufs=1) as cpool,
            tc.tile_pool(name="xin", bufs=4) as xpool,
            tc.tile_pool(name="zbuf", bufs=1) as zpool,
            tc.tile_pool(name="wts", bufs=3) as wpool,
            tc.tile_pool(name="outs", bufs=4) as opool,
            tc.tile_pool(name="ps1", bufs=2, space="PSUM") as ps1,
            tc.tile_pool(name="psmm", bufs=4, space="PSUM") as psmm,
        ):
            h_t = cpool.tile([P, P], bf, tag="h")
            nc.sync.dma_start(h_t, hh)
            sb_t = cpool.tile([P, O_CH], f32, tag="sb")
            nc.sync.dma_start(sb_t, sb)
            bb_t = cpool.tile([P, O_CH], f32, tag="bb")
            nc.sync.dma_start(bb_t, bb)

            def stage1(mh, ping):
                msl = slice(mh * M_HALF, (mh + 1) * M_HALF)
                for a in range(A_CH):
                    x_t = xpool.tile([P, M_HALF], bf, tag="x")
                    nc.sync.dma_start(x_t, xT[a, :, msl])
                    ps = ps1.tile([P, M_HALF], f32, tag="s1")
                    nc.tensor.matmul(ps, h_t, x_t, start=True, stop=True)
                    nc.scalar.copy(ping[:, a, :], ps)

            def butterflies(ping, pong):
                src, dst = ping, pong
                for st in range(5):
                    hstep = 1 << st
                    for i in range(A_CH):
                        if i & hstep:
                            continue
                        j = i + hstep
                        nc.vector.tensor_add(dst[:, i, :], src[:, i, :], src[:, j, :])
                        nc.vector.tensor_sub(dst[:, j, :], src[:, i, :], src[:, j, :])
                    src, dst = dst, src
                return src  # buffer holding the final transform

            def gemm(mh, zfin):
                msl = slice(mh * M_HALF, (mh + 1) * M_HALF)
                for ot in range(O_CH):
                    w_t = wpool.tile([P, A_CH, P], bf, tag="w")
                    nc.sync.dma_start(w_t, wT[ot])
                    ps = psmm.tile([P, M_HALF], f32, tag="mm")
                    for k in range(A_CH):
                        nc.tensor.matmul(ps, w_t[:, k, :], zfin[:, k, :],
                                         start=(k == 0), stop=(k == A_CH - 1))
                    o_t = opool.tile([P, M_HALF], f32, tag="o")
                    nc.scalar.activation(o_t, ps, IDENT,
                                         bias=bb_t[:, ot:ot + 1],
                                         scale=sb_t[:, ot:ot + 1])
                    nc.sync.dma_start(yT[ot, :, msl], o_t)

            for _ in range(rep):
                zfin = []
                for mh in range(2):
                    ping = zpool.tile([P, A_CH, M_HALF], bf, tag="ping")
                    pong = zpool.tile([P, A_CH, M_HALF], bf, tag=f"pong{mh}")
                    stage1(mh, ping)
                    zfin.append(butterflies(ping, pong))
                for mh in range(2):
                    gemm(mh, zfin[mh])

    nc.compile()
    return nc


def _prep_shared(Q, s, bias):
    """Host-side packaging of the replicated operands (layout + dtype only)."""
    Q = np.asarray(Q)
    # Q arrives int8 (reference setup) or int32 (spec); both exact in bf16.
    q4 = Q.reshape(O_CH, P, A_CH, P)            # [ot, o, k, p]
    w_host = np.ascontiguousarray(
        q4.transpose(0, 3, 2, 1)).astype(BF16)  # [ot, p, k, o]
    s_eff = (np.asarray(s, dtype=np.float32).reshape(OUT_F) / 64.0)
    sb_host = np.ascontiguousarray(s_eff.reshape(O_CH, P).T)         # [p, ot]
    bb_host = np.ascontiguousarray(
        np.asarray(bias, dtype=np.float32).reshape(O_CH, P).T)       # [p, ot]
    h_host = _sylvester(7).astype(BF16)                              # [128,128]
    return w_host, sb_host, bb_host, h_host


def _make_in_maps(x, Q, s, bias):
    x = np.asarray(x, dtype=np.float32)
    w_host, sb_host, bb_host, h_host = _prep_shared(Q, s, bias)
    x_flat = x.reshape(M_TOT, IN_F).astype(BF16)
    in_maps = []
    for c in range(N_CORES):
        shard = x_flat[c * M_CORE:(c + 1) * M_CORE]              # [1024, 4096]
        xT_host = np.ascontiguousarray(shard.T).reshape(A_CH, P, M_CORE)
        in_maps.append({
            "xT": xT_host,
            "wT": w_host,
            "sb": sb_host,
            "bb": bb_host,
            "h": h_host,
        })
    return in_maps


def _assemble_output(results):
    y = np.empty((M_TOT, OUT_F), dtype=np.float32)
    for c in range(N_CORES):
        yT_core = results[c]["yT"]                               # [32,128,1024]
        y[c * M_CORE:(c + 1) * M_CORE] = (
            yT_core.transpose(2, 0, 1).reshape(M_CORE, OUT_F))
    return y.reshape(B, S, OUT_F)


def kernel(x, Q, s, bias):
    global LAST_RESULTS
    from concourse.bass_utils import run_bass_kernel_spmd

    if "nc" not in _CACHE:
        _CACHE["nc"] = _build_nc()
    nc = _CACHE["nc"]

    in_maps = _make_in_maps(x, Q, s, bias)
    res = run_bass_kernel_spmd(nc, in_maps, core_ids=list(range(N_CORES)))
    LAST_RESULTS = res
    return _assemble_output(res.results)


# revision 26
# speedup vs baseline: 5.8629x; 1.2461x over previous
"""Trainium2 Bass kernel for DTDRLinear.

Reference computation:
    y = hadamard(x) @ (Q.astype(f32) * s).T + bias        x:[4,2048,4096]

Key algebra: the 4096-point normalized Walsh-Hadamard transform is the
symmetric Sylvester matrix H4096/64, and H4096 = H32 (x) H128 (Kronecker,
index i = a*128 + b).  So with x kept transposed (in_f on partitions):

    z = H4096 @ x^T      (unnormalized)
      = (stage 1)  per 128-row chunk a:  u_a = H128 @ xT_a     -> PE matmul
        (stage 2)  5 butterfly stages over the 32 chunk indices -> DVE adds
    y^T[o,:] = (s[o]/64) * (Q[o,:] @ z) + bias[o]              -> PE + ACT

Sharding: data-parallel over the 8192 tokens across 8 cores (1024 each);
Q/s/bias replicated.  The dequant scale s/64 and bias are fused into the
PSUM->SBUF eviction on the Scalar engine, so the weights used on the PE are
the raw int8 values of Q exactly represented in bf16.
"""

import math
import os
import sys

import numpy as np

for _p in ("/opt/trn_rl_repo", os.path.expanduser("~/.axon_site/_ro/trn_rl_repo")):
    if os.path.isdir(_p) and _p not in sys.path:
        sys.path.insert(0, _p)

import ml_dtypes  # noqa: E402

BF16 = ml_dtypes.bfloat16

# Problem shape (hardcoded per the task contract).
B, S, IN_F, OUT_F = 4, 2048, 4096, 4096
N_CORES = 8
M_TOT = B * S                    # 8192 tokens
M_CORE = M_TOT // N_CORES        # 1024 tokens per core
M_HALF = M_CORE // 2             # 512-token halves pipelined through SBUF
P = 128
A_CH = IN_F // P                 # 32 chunks of 128 along in_f
O_CH = OUT_F // P                # 32 tiles of 128 along out_f

_CACHE: dict = {}
LAST_RESULTS = None              # BassKernelResults from the most recent run


def _sylvester(k: int) -> np.ndarray:
    H = np.array([[1.0]], dtype=np.float32)
    for _ in range(k):
        H = np.block([[H, H], [H, -H]])
    return H


def _build_nc(rep: int = 1):
    """Build the SPMD per-core program. rep>1 repeats the whole compute body
    (same inputs, overwriting outputs) for amplified wall-clock timing."""
    import concourse.mybir as mybir
    import concourse.tile as tile
    from concourse import bacc

    bf = mybir.dt.bfloat16
    f32 = mybir.dt.float32
    IDENT = mybir.ActivationFunctionType.Identity

    nc = bacc.Bacc("TRN2", target_bir_lowering=False, debug=False,
                   enable_asserts=False, enable_partition_id=False)

    xT = nc.dram_tensor("xT", [A_CH, P, M_CORE], bf, kind="ExternalInput").ap()
    wT = nc.dram_tensor("wT", [O_CH, P, A_CH, P], bf, kind="ExternalInput").ap()
    sb = nc.dram_tensor("sb", [P, O_CH], f32, kind="ExternalInput").ap()
    bb = nc.dram_tensor("bb", [P, O_CH], f32, kind="ExternalInput").ap()
    hh = nc.dram_tensor("h", [P, P], bf, kind="ExternalInput").ap()
    yT = nc.dram_tensor("yT", [O_CH, P, M_CORE], f32, kind="ExternalOutput").ap()

    with tile.TileContext(nc) as tc:
        with (
            tc.tile_pool(name="const", bufs=1) as cpool,
            tc.tile_pool(name="xin", bufs=8) as xpool,
            tc.tile_pool(name="zbuf", bufs=1) as zpool,
            tc.tile_pool(name="zf", bufs=2) as zfpool,
            tc.tile_pool(name="wts", bufs=4) as wpool,
            tc.tile_pool(name="outs", bufs=6) as opool,
            tc.tile_pool(name="ps1", bufs=2, space="PSUM") as ps1,
            tc.tile_pool(name="psmm", bufs=6, space="PSUM") as psmm,
        ):
            h_t = cpool.tile([P, P], bf, tag="h")
            nc.sync.dma_start(h_t, hh)
            sb_t = cpool.tile([P, O_CH], f32, tag="sb")
            nc.sync.dma_start(sb_t, sb)
            bb_t = cpool.tile([P, O_CH], f32, tag="bb")
            nc.sync.dma_start(bb_t, bb)

            # Pipeline slices (each <=512 tokens) grouped into weight passes:
            # "256|384,384" = pass A over a 256-token slice (fast GEMM start
            # while later slices transform), pass B over two 384 slices
            # sharing one weight stream.
            GROUPS = [
                [int(t) for t in g.split(",")]
                for g in os.environ.get("KERNEL_GROUPS", "512|512").split("|")
            ]
            SEGS = [ln for g in GROUPS for ln in g]
            assert sum(SEGS) == M_CORE and max(SEGS) <= 512
            SEG_OFF = [sum(SEGS[:i]) for i in range(len(SEGS))]
            SEG_MAX = max(SEGS)
            N_SL = len(SEGS)
            # k order matching butterfly stage-5 completion: pair (i, i+16)
            # finalizes at stage-5 op i.
            K_ORDER = [k for i in range(A_CH // 2) for k in (i, i + A_CH // 2)]

            def stage1(si, ping):
                off, ln = SEG_OFF[si], SEGS[si]
                for a in range(A_CH):
                    x_t = xpool.tile([P, SEG_MAX], bf, tag="x", name="x_t")[:, :ln]
                    nc.sync.dma_start(x_t, xT[a, :, off:off + ln])
                    ps = ps1.tile([P, SEG_MAX], f32, tag="s1", name="ps_s1")[:, :ln]
                    nc.tensor.matmul(ps, h_t, x_t, start=True, stop=True)
                    nc.scalar.copy(ping[:, a, :], ps)

            def butterflies(ping, pong, ln):
                # Fine-grained per-pair ops pipeline with stage-1 evictions
                # (coarse fused ops would serialize on the whole ping buffer).
                # The last stage writes 32 separate tiles so the GEMM's
                # per-chunk reads get op-granular dependencies.
                src, dst = ping, pong
                for st in range(4):
                    hstep = 1 << st
                    for i in range(A_CH):
                        if i & hstep:
                            continue
                        j = i + hstep
                        nc.vector.tensor_add(dst[:, i, :], src[:, i, :], src[:, j, :])
                        nc.vector.tensor_sub(dst[:, j, :], src[:, i, :], src[:, j, :])
                    src, dst = dst, src
                zf = [zfpool.tile([P, SEG_MAX], bf, tag=f"zf{i}",
                                  name=f"zf_{i}", bufs=min(3, N_SL))[:, :ln]
                      for i in range(A_CH)]
                half = A_CH // 2
                for i in range(half):
                    j = i + half
                    nc.vector.tensor_add(zf[i], src[:, i, :], src[:, j, :])
                    nc.vector.tensor_sub(zf[j], src[:, i, :], src[:, j, :])
                return zf

            def gemm(slices, zfins):
                """One weight pass covering the given slice indices."""
                for ot in range(O_CH):
                    w_t = wpool.tile([P, A_CH, P], bf, tag="w")
                    nc.sync.dma_start(w_t, wT[ot])
                    for si, zfin in zip(slices, zfins):
                        off, ln = SEG_OFF[si], SEGS[si]
                        ps = psmm.tile([P, SEG_MAX], f32, tag="mm",
                                       name="ps_mm")[:, :ln]
                        for n, k in enumerate(K_ORDER):
                            nc.tensor.matmul(ps, w_t[:, k, :], zfin[k],
                                             start=(n == 0), stop=(n == A_CH - 1))
                        o_t = opool.tile([P, SEG_MAX], f32, tag="o",
                                         name="o_t")[:, :ln]
                        nc.scalar.activation(o_t, ps, IDENT,
                                             bias=bb_t[:, ot:ot + 1],
                                             scale=sb_t[:, ot:ot + 1])
                        nc.sync.dma_start(yT[ot, :, off:off + ln], o_t)

            for _ in range(rep):
                zfin = []
                for si in range(N_SL):
                    ping = zpool.tile([P, A_CH, SEGS[si]], bf, tag="ping")
                    pong = zpool.tile([P, A_CH, SEGS[si]], bf, tag="pong")
                    stage1(si, ping)
                    zfin.append(butterflies(ping, pong, SEGS[si]))
                si0 = 0
                for g in GROUPS:
                    idxs = list(range(si0, si0 + len(g)))
                    gemm(idxs, [zfin[i] for i in idxs])
                    si0 += len(g)

    nc.compile()
    return nc


def _prep_shared(Q, s, bias):
    """Host-side packaging of the replicated operands (layout + dtype only)."""
    Q = np.asarray(Q)
    # Q arrives int8 (reference setup) or int32 (spec); both exact in bf16.
    q4 = Q.reshape(O_CH, P, A_CH, P)            # [ot, o, k, p]
    w_host = np.ascontiguousarray(
        q4.transpose(0, 3, 2, 1)).astype(BF16)  # [ot, p, k, o]
    s_eff = (np.asarray(s, dtype=np.float32).reshape(OUT_F) / 64.0)
    sb_host = np.ascontiguousarray(s_eff.reshape(O_CH, P).T)         # [p, ot]
    bb_host = np.ascontiguousarray(
        np.asarray(bias, dtype=np.float32).reshape(O_CH, P).T)       # [p, ot]
    h_host = _sylvester(7).astype(BF16)                              # [128,128]
    return w_host, sb_host, bb_host, h_host


def _make_in_maps(x, Q, s, bias):
    x = np.asarray(x, dtype=np.float32)
    w_host, sb_host, bb_host, h_host = _prep_shared(Q, s, bias)
    x_flat = x.reshape(M_TOT, IN_F).astype(BF16)
    in_maps = []
    for c in range(N_CORES):
        shard = x_flat[c * M_CORE:(c + 1) * M_CORE]              # [1024, 4096]
        xT_host = np.ascontiguousarray(shard.T).reshape(A_CH, P, M_CORE)
        in_maps.append({
            "xT": xT_host,
            "wT": w_host,
            "sb": sb_host,
            "bb": bb_host,
            "h": h_host,
        })
    return in_maps


def _assemble_output(results):
    y = np.empty((M_TOT, OUT_F), dtype=np.float32)
    for c in range(N_CORES):
        yT_core = results[c]["yT"]                               # [32,128,1024]
        y[c * M_CORE:(c + 1) * M_CORE] = (
            yT_core.transpose(2, 0, 1).reshape(M_CORE, OUT_F))
    return y.reshape(B, S, OUT_F)


def kernel(x, Q, s, bias):
    global LAST_RESULTS
    from concourse.bass_utils import run_bass_kernel_spmd

    if "nc" not in _CACHE:
        _CACHE["nc"] = _build_nc()
    nc = _CACHE["nc"]

    in_maps = _make_in_maps(x, Q, s, bias)
    res = run_bass_kernel_spmd(nc, in_maps, core_ids=list(range(N_CORES)))
    LAST_RESULTS = res
    return _assemble_output(res.results)
